# revision 33
# baseline (speedup 1.0000x reference)
"""Trainium2 Bass kernel for nn_Classical_autoencoder (patch MLP autoencoder + cosine fold).

Contract: kernel(**inputs) takes FULL inputs (img (32,1,512,512), W1 (16,4), b1 (4,),
W2 (4,4), b2 (4,), W3 (4,16), b3 (16,)) and returns the FULL (32,512,512) output.
Internally: pure data-parallel over 8 NeuronCores, 4 images per core.

Math (per image):
  patches x = im2col(img, 4x4, stride 2)           # (255*255, 16)
  y = relu(relu(relu(x@W1+b1)@W2+b2)@W3+b3)        # (P, 16)
  S[i,j] = x.y / (max(|x|,eps)*max(|y|,eps))       # (255,255)
  out[r,c] = mean of S[i,j] for i in {r//2-1, r//2} & [0,255), j likewise
  (the overlapping fold with k=4,s=2 reduces exactly to this 2-tap box filter
   on S, upsampled 2x with 2x2-constant blocks)

Layout on chip (per image):
  X [128=(32k+g), t(2), ci(4), n(512)] bf16 where n = 2*jj+li2 holds
      img[16g+k+4ci+2li2, 2jj+t]; patch row i = 8g+2ci+li2, channel (k,l),
      l=(t=l%2, dl=l//2): element at [32k+g, t, ci, n+2dl].
  MLP runs with patches as matmul free dim (510 columns = one PSUM bank per
  matmul). Dot-product contractions: per-l products are pre-reduced over l
  (3 bf16 adds on DVE/Pool), then one 128->128 matmul per ci (cw4, ones
  block) contracts k; outputs land at partition 4g+ci = patch-row-pair
  index, so the similarity tile is directly in the fold's layout (no DRAM
  bounce). rsqrt = sqrt activation + reciprocal_approx_fast.
"""

import sys

for _p in ("/opt/trn_rl_repo", "/root/.axon_site/_ro/trn_rl_repo"):
    if _p not in sys.path:
        sys.path.append(_p)

from contextlib import ExitStack

import numpy as np

import concourse.bass as bass
import concourse.tile as tile
from concourse import bacc, mybir

F32 = mybir.dt.float32
BF16 = mybir.dt.bfloat16

IMG = 512
KS = 4
STRIDE = 2
OH = 255  # output patches per dim
NSAMP = 4  # images per core
NCORES = 8

MULT = mybir.AluOpType.mult
ADD = mybir.AluOpType.add
MAX = mybir.AluOpType.max
RELU = mybir.ActivationFunctionType.Relu
SQRT = mybir.ActivationFunctionType.Sqrt
COPY = mybir.ActivationFunctionType.Copy


def build_nc() -> bass.Bass:
    nc = bacc.Bacc()

    img4b = nc.declare_dram_parameter("img4b", [NSAMP, 128, 8 * IMG], BF16, isOutput=False)[:]
    imgsq = nc.declare_dram_parameter("imgsq", [NSAMP, 128, 4 * IMG], BF16, isOutput=False)[:]
    bandw = nc.declare_dram_parameter("bandw", [128, 8, 128], BF16, isOutput=False)[:]
    l1w = nc.declare_dram_parameter("l1w", [128, 4, 128], BF16, isOutput=False)[:]
    l2w = nc.declare_dram_parameter("l2w", [128, 128], BF16, isOutput=False)[:]
    l3w = nc.declare_dram_parameter("l3w", [128, 4, 128], BF16, isOutput=False)[:]
    b3v = nc.declare_dram_parameter("b3v", [128, 4], F32, isOutput=False)[:]
    cw4 = nc.declare_dram_parameter("cw4", [128, 4, 128], BF16, isOutput=False)[:]
    b1v = nc.declare_dram_parameter("b1v", [128, 1], F32, isOutput=False)[:]
    b2v = nc.declare_dram_parameter("b2v", [128, 1], F32, isOutput=False)[:]
    out4 = nc.declare_dram_parameter("out4", [NSAMP, IMG, IMG], F32, isOutput=True)[:]

    with ExitStack() as ctx:
        tc = ctx.enter_context(tile.TileContext(nc))
        consts = ctx.enter_context(tc.tile_pool(name="consts", bufs=1))
        xp = ctx.enter_context(tc.tile_pool(name="xp", bufs=2))
        sqp = ctx.enter_context(tc.tile_pool(name="sqp", bufs=2))
        yvp = ctx.enter_context(tc.tile_pool(name="yvp", bufs=2))
        hp = ctx.enter_context(tc.tile_pool(name="hp", bufs=2))
        prp = ctx.enter_context(tc.tile_pool(name="prp", bufs=2))
        sadd = ctx.enter_context(tc.tile_pool(name="sadd", bufs=1))
        s3p = ctx.enter_context(tc.tile_pool(name="s3p", bufs=2))
        simp = ctx.enter_context(tc.tile_pool(name="simp", bufs=1))
        foldp = ctx.enter_context(tc.tile_pool(name="foldp", bufs=2))
        upp = ctx.enter_context(tc.tile_pool(name="upp", bufs=1))
        psz = ctx.enter_context(tc.tile_pool(name="psz", bufs=1, space="PSUM"))
        psct = ctx.enter_context(tc.tile_pool(name="psct", bufs=1, space="PSUM"))

        # ---- constants ----
        l1w_t = consts.tile([128, 4, 128], BF16)
        nc.sync.dma_start(out=l1w_t, in_=l1w[:, :, :])
        l2w_t = consts.tile([128, 128], BF16)
        nc.sync.dma_start(out=l2w_t, in_=l2w[:, :])
        l3w_t = consts.tile([128, 4, 128], BF16)
        nc.sync.dma_start(out=l3w_t, in_=l3w[:, :, :])
        b3_t = consts.tile([128, 4], F32)
        nc.sync.dma_start(out=b3_t, in_=b3v[:, :])
        cw4_t = consts.tile([128, 4, 128], BF16)
        nc.sync.dma_start(out=cw4_t, in_=cw4[:, :, :])
        bandw_t = consts.tile([128, 8, 128], BF16)
        nc.sync.dma_start(out=bandw_t, in_=bandw[:, :, :])
        b1_t = consts.tile([128, 1], F32)
        nc.sync.dma_start(out=b1_t, in_=b1v[:, :])
        b2_t = consts.tile([128, 1], F32)
        nc.sync.dma_start(out=b2_t, in_=b2v[:, :])
        eps_t = consts.tile([128, 1], F32)
        nc.vector.memset(eps_t, 1e-20)

        def xv(base, l, ci):
            t, dl = l % 2, l // 2
            return base[:, t, ci, 2 * dl : 2 * dl + 510]

        def xva(base, l):
            t, dl = l % 2, l // 2
            return base[:, t, :, 2 * dl : 2 * dl + 510]

        def emit_front(s):
            """DMA + MLP + products + pre-reduce for image s."""
            # ---- input: X in the (t, ci, n) layout; one contiguous DMA ----
            X = xp.tile([128, 2, 4, 512], BF16, tag="x", name=f"X_{s}")
            nc.sync.dma_start(
                out=X, in_=img4b[s].rearrange("p (t c n) -> p t c n", t=2, c=4)
            )

            # |x|^2 path: host-squared natural-layout image; column box-sum on
            # DVE/Pool, then 4-row band-matrix sums on the PE into ctx (tail).
            sqt = sqp.tile([128, 4, 512], BF16, tag="sq", name=f"sqt_{s}")
            nc.sync.dma_start(
                out=sqt, in_=imgsq[s].rearrange("p (t c) -> p t c", t=4)
            )
            sqr = sqt.rearrange("p t (j two) -> p t two j", two=2)
            t1 = sqp.tile([128, 4, 256], BF16, tag="t1", name=f"t1_{s}")
            nc.gpsimd.tensor_tensor(t1, sqr[:, :, 0, :], sqr[:, :, 1, :], ADD)
            bb = sqp.tile([128, 4, 255], BF16, tag="bb", name=f"bb_{s}")
            nc.gpsimd.tensor_tensor(bb, t1[:, :, 0:255], t1[:, :, 1:256], ADD)

            yv = yvp.tile([128, 4, 4, 512], BF16, tag="yv", name=f"yv_{s}")

            # ---- layer 1 (l outer; pair-granularity PSUM tiles, 2 banks each) ----
            z1p = [
                psz.tile([128, 2, 512], F32, tag=f"z{P}", name=f"z1p{P}_{s}")
                for P in range(2)
            ]
            for l in range(4):
                for P in range(2):
                    for ci2 in range(2):
                        nc.tensor.matmul(
                            z1p[P][:, ci2, 0:510], l1w_t[:, l, :],
                            xv(X, l, 2 * P + ci2),
                            start=(l == 0), stop=(l == 3),
                        )
            h1 = []
            for P in range(2):
                h = hp.tile([128, 2, 510], BF16, tag=f"h1{P}", name=f"h1_{P}_{s}")
                nc.scalar.activation(h, z1p[P][:, :, 0:510], RELU, bias=b1_t[:, :])
                h1.append(h)
            # ---- layer 2 ----
            z2p = [
                psz.tile([128, 2, 512], F32, tag=f"z{P}", name=f"z2p{P}_{s}")
                for P in range(2)
            ]
            for P in range(2):
                for ci2 in range(2):
                    nc.tensor.matmul(
                        z2p[P][:, ci2, 0:510], l2w_t[:, :], h1[P][:, ci2, :],
                        start=True, stop=True,
                    )
            h2 = []
            for P in range(2):
                h = hp.tile([128, 2, 510], BF16, tag=f"h2{P}", name=f"h2_{P}_{s}")
                nc.scalar.activation(h, z2p[P][:, :, 0:510], RELU, bias=b2_t[:, :])
                h2.append(h)
            # ---- layer 3: z3 + bias-relu into yv chunks ----
            pr = prp.tile([128, 4, 4, 510], BF16, tag="pr", name=f"pr_{s}")
            for l in range(4):
                for P in range(2):
                    z3 = psz.tile(
                        [128, 2, 512], F32, tag=f"z{P}", name=f"z3_{l}_{P}_{s}"
                    )
                    for ci2 in range(2):
                        nc.tensor.matmul(
                            z3[:, ci2, 0:510], l3w_t[:, l, :], h2[P][:, ci2, :],
                            start=True, stop=True,
                        )
                    nc.scalar.activation(
                        yv[:, 2 * P : 2 * P + 2, l, 0:510], z3[:, :, 0:510],
                        RELU, bias=b3_t[:, l : l + 1],
                    )
                # x*y products for this l over all ci (vector)
                nc.vector.tensor_tensor(pr[:, l], xva(X, l), yv[:, :, l, 0:510], MULT)
            # y^2 in two halves (vector; pool is too slow and contends on SBUF)
            yq = prp.tile([128, 4, 4, 512], BF16, tag="yq", name=f"yq_{s}")
            nc.vector.tensor_tensor(yq[:, 0:2], yv[:, 0:2], yv[:, 0:2], MULT)
            nc.vector.tensor_tensor(yq[:, 2:4], yv[:, 2:4], yv[:, 2:4], MULT)

            # ---- pre-reduce over l (bf16 partial sums, interleaved layout) ----
            s3 = s3p.tile([128, 2, 4, 510], BF16, tag="s3", name=f"s3_{s}")
            with nc.allow_low_precision(reason="bf16 partial dot sums; 2e-2 budget"):
                # dsum (vector)
                da = sadd.tile([128, 4, 510], BF16, tag="da", name=f"da_{s}")
                nc.vector.tensor_tensor(da, pr[:, 0], pr[:, 1], ADD)
                db = sadd.tile([128, 4, 510], BF16, tag="db", name=f"db_{s}")
                nc.vector.tensor_tensor(db, pr[:, 2], pr[:, 3], ADD)
                nc.vector.tensor_tensor(s3[:, 0], da, db, ADD)
                # ysum (pool does the first add)
                ya = sadd.tile([128, 4, 510], BF16, tag="ya", name=f"ya_{s}")
                nc.gpsimd.tensor_tensor(ya, yq[:, :, 0, 0:510], yq[:, :, 1, 0:510], ADD)
                yb = sadd.tile([128, 4, 510], BF16, tag="yb", name=f"yb_{s}")
                nc.vector.tensor_tensor(yb, yq[:, :, 2, 0:510], yq[:, :, 3, 0:510], ADD)
                nc.vector.tensor_tensor(s3[:, 1], ya, yb, ADD)
            return bb, s3

        def emit_tail(s, bb, s3):
            """Contractions + similarity + fold + output for image s."""
            ctd = psct.tile([128, 2, 255], F32, tag="ctd", name=f"ctd_{s}")
            ctx_ = psct.tile([128, 2, 255], F32, tag="ctx", name=f"ctx_{s}")
            cty = psct.tile([128, 2, 255], F32, tag="cty", name=f"cty_{s}")

            # ---- band matmuls: ctx[q',e,:] = sum_k B[4q'+2e+k,:] over 4 tiles ----
            for e in range(2):
                for t in range(4):
                    nc.tensor.matmul(
                        ctx_[:, e, :], bandw_t[:, 2 * t + e, :], bb[:, t, :],
                        start=(t == 0), stop=(t == 3),
                    )

            # ---- contract k: one matmul per (ci, quantity); out partition 4g+ci,
            # moving AP deinterleaves (li2-major cols) so ct/sim/fold layouts match.
            for ci in range(4):
                for q in range(2):
                    nc.tensor.matmul(
                        (ctd, cty)[q],
                        cw4_t[:, ci, :],
                        s3[:, q, ci, :].rearrange("p (j l) -> p l j", l=2),
                        start=(ci == 0), stop=(ci == 3),
                    )

            # ---- cosine similarity (scaled by 1/4 for the fold) ----
            # partition q = 4g+ci = patch-row-pair index (rows 2q+li2): the
            # similarity tile is directly in the fold's row-pair layout.
            ctxs = simp.tile([128, 2, 255], BF16, tag="ctxs")
            nc.scalar.activation(ctxs, ctx_, COPY)
            m_ = simp.tile([128, 2, 255], BF16, tag="m")
            nc.vector.tensor_tensor(m_, ctxs, cty, MULT)
            sq = simp.tile([128, 2, 255], F32, tag="sq")
            nc.scalar.activation(sq, m_, SQRT, bias=eps_t[:, :], scale=16.0)
            r_ = simp.tile([128, 2, 255], F32, tag="r")
            nc.vector.reciprocal_approx_fast(r_, sq)
            simt = foldp.tile([128, 2, 255], BF16, tag="simt")
            nc.vector.tensor_tensor(simt, ctd, r_, MULT)

            # ---- fold (bf16): R[i,v] = S[i,v-1]+S[i,v], with edge doubling ----
            with nc.allow_low_precision(reason="bf16 fold sums; 2e-2 budget"):
                rf = foldp.tile([128, 2, 256], BF16, tag="rf")
                nc.vector.tensor_tensor(
                    rf[:, :, 1:255], simt[:, :, 0:254], simt[:, :, 1:255], ADD
                )
                nc.scalar.activation(rf[:, :, 0:1], simt[:, :, 0:1], COPY, scale=2.0)
                nc.scalar.activation(
                    rf[:, :, 255:256], simt[:, :, 254:255], COPY, scale=2.0
                )
                # S row 255 doesn't exist -> duplicate row 254 so T[255]=2*R[254]
                nc.sync.dma_start(out=rf[127:128, 1, :], in_=rf[127:128, 0, :])
                # partition-shifted copy of odd rows: rfs[q] = R[2q-1] (rfs[0]=R[0])
                rfs = foldp.tile([128, 256], BF16, tag="rfs")
                nc.sync.dma_start(out=rfs[1:128, :], in_=rf[0:127, 1, :])
                nc.sync.dma_start(out=rfs[0:1, :], in_=rf[0:1, 0, :])
                tf = foldp.tile([128, 2, 256], BF16, tag="tf")
                nc.vector.tensor_tensor(tf[:, 1, :], rf[:, 0, :], rf[:, 1, :], ADD)
                nc.vector.tensor_tensor(tf[:, 0, :], rfs, rf[:, 0, :], ADD)

            # ---- upsample 2x2 (bf16 -> f32 via copy) and store ----
            up = upp.tile([128, 2, 2, 512], F32, tag="up")  # (lu, ru, c=2v+cv)
            upr = up.rearrange("p lu ru (v cv) -> p lu ru cv v", cv=2)
            for ru in range(2):
                for cv in range(2):
                    eng = nc.gpsimd if (ru == 0 and cv == 0) else nc.vector
                    eng.tensor_copy(upr[:, :, ru, cv, :], tf[:, :, :])
            nc.sync.dma_start(
                out=bass.AP(
                    tensor=out4.tensor,
                    offset=out4.offset + s * IMG * IMG,
                    ap=[[4 * IMG, 128], [1, 4 * IMG]],
                ),
                in_=up,
            )

        # Software pipeline: each image's contraction/similarity/fold block is
        # emitted after the NEXT image's MLP so the PE never waits on the
        # vector-engine pre-reduce of the image it just fed.
        pending = None
        for s in range(NSAMP):
            front = emit_front(s)
            if pending is not None:
                emit_tail(pending[0], *pending[1])
            pending = (s, front)
        emit_tail(pending[0], *pending[1])

    nc.finalize()
    return nc


def make_weight_inputs(W1, b1, W2, b2, W3, b3):
    """Host-side block-diagonal weight construction (all fp32)."""
    W1 = np.asarray(W1, np.float32)
    W2 = np.asarray(W2, np.float32)
    W3 = np.asarray(W3, np.float32)
    b1 = np.asarray(b1, np.float32)
    b2 = np.asarray(b2, np.float32)
    b3 = np.asarray(b3, np.float32)
    # partition orders: image/z3 rows p = 32k+g ; h1/h2 rows q = 32c+g
    l1w = np.zeros((128, 4, 128), np.float32)
    l2w = np.zeros((128, 128), np.float32)
    l3w = np.zeros((128, 4, 128), np.float32)
    b3v = np.zeros((128, 4), np.float32)
    cw4m = np.zeros((128, 4, 128), np.float32)
    for g in range(32):
        for l in range(4):
            for k in range(4):
                for c in range(4):
                    l1w[32 * k + g, l, 32 * c + g] = W1[4 * k + l, c]
                    l3w[32 * c + g, l, 32 * k + g] = W3[c, 4 * k + l]
                b3v[32 * k + g, l] = b3[4 * k + l]
            for ci in range(4):
                # ct output partition = 4g+ci = patch-row-pair index
                cw4m[32 * l + g, ci, 4 * g + ci] = 1.0
        for c in range(4):
            for c2 in range(4):
                l2w[32 * c + g, 32 * c2 + g] = W2[c, c2]
    b1v = np.repeat(b1, 32).reshape(128, 1).astype(np.float32)
    b2v = np.repeat(b2, 32).reshape(128, 1).astype(np.float32)
    # band weights: bandw[r, 2t+e, m] = 1 iff 4m+2e <= 128t+r <= 4m+2e+3
    bandwm = np.zeros((128, 8, 128), np.float32)
    r = np.arange(128)[:, None]
    m = np.arange(128)[None, :]
    for t in range(4):
        for e in range(2):
            R = 128 * t + r
            bandwm[:, 2 * t + e, :] = (
                (4 * m + 2 * e <= R) & (R <= 4 * m + 2 * e + 3)
            ).astype(np.float32)
    import ml_dtypes

    bf = ml_dtypes.bfloat16
    return {
        "l1w": l1w.astype(bf), "l2w": l2w.astype(bf), "l3w": l3w.astype(bf),
        "b3v": b3v, "cw4": cw4m.astype(bf), "b1v": b1v, "b2v": b2v,
        "bandw": bandwm.astype(bf),
    }


_NC = None


def get_nc():
    global _NC
    if _NC is None:
        _NC = build_nc()
    return _NC


def _bf16():
    import ml_dtypes

    return ml_dtypes.bfloat16


def gather_rows(img_n):
    """(n,512,512) f32 -> (n,128,4096) bf16: X[p,t,ci,jj,li2] = img[16g+k+4ci+2li2, 2jj+t]."""
    n = img_n.shape[0]
    pad = np.zeros((n, IMG + 2, IMG), np.float32)
    pad[:, :IMG, :] = img_n
    p = np.arange(128)
    g, k = p % 32, p // 32
    ci = np.arange(4)
    li2 = np.arange(2)
    t = np.arange(2)
    jj = np.arange(256)
    rows = (
        (16 * g + k)[:, None, None, None, None]
        + 4 * ci[None, None, :, None, None]
        + 2 * li2[None, None, None, None, :]
    )
    cols = (2 * jj[None, None, None, :, None] + t[None, :, None, None, None])
    rows = np.broadcast_to(rows, (128, 2, 4, 256, 2))
    cols = np.broadcast_to(cols, (128, 2, 4, 256, 2))
    out = pad[:, rows, cols]  # (n,128,2,4,256,2)
    return np.ascontiguousarray(out.reshape(n, 128, 8 * IMG)).astype(_bf16())


def gather_sq(img_n):
    """(n,512,512) f32 -> (n,128,2048) bf16: imgsq[p,t,col] = img[128t+p, col]^2."""
    n = img_n.shape[0]
    sq = (img_n.astype(np.float32) ** 2).reshape(n, 4, 128, IMG)
    return np.ascontiguousarray(
        sq.transpose(0, 2, 1, 3).reshape(n, 128, 4 * IMG)
    ).astype(_bf16())


def kernel(img, W1, b1, W2, b2, W3, b3):
    from concourse.bass_utils import run_bass_kernel_spmd

    img = np.asarray(img, np.float32).reshape(32, IMG, IMG)
    wts = make_weight_inputs(W1, b1, W2, b2, W3, b3)
    nc = get_nc()
    core_ids = list(range(NCORES))
    in_maps = []
    for c in range(NCORES):
        m = {
            "img4b": gather_rows(img[c * NSAMP : (c + 1) * NSAMP]),
            "imgsq": gather_sq(img[c * NSAMP : (c + 1) * NSAMP]),
        }
        m.update(wts)
        in_maps.append(m)
    res = run_bass_kernel_spmd(nc, in_maps, core_ids)
    out = np.concatenate([res.results[i]["out4"] for i in range(NCORES)], axis=0)
    return out.astype(np.float32)


# revision 34
# speedup vs baseline: 1.0248x; 1.0248x over previous
"""Trainium2 Bass kernel for nn_Classical_autoencoder (patch MLP autoencoder + cosine fold).

Contract: kernel(**inputs) takes FULL inputs (img (32,1,512,512), W1 (16,4), b1 (4,),
W2 (4,4), b2 (4,), W3 (4,16), b3 (16,)) and returns the FULL (32,512,512) output.
Internally: pure data-parallel over 8 NeuronCores, 4 images per core.

Math (per image):
  patches x = im2col(img, 4x4, stride 2)           # (255*255, 16)
  y = relu(relu(relu(x@W1+b1)@W2+b2)@W3+b3)        # (P, 16)
  S[i,j] = x.y / (max(|x|,eps)*max(|y|,eps))       # (255,255)
  out[r,c] = mean of S[i,j] for i in {r//2-1, r//2} & [0,255), j likewise
  (the overlapping fold with k=4,s=2 reduces exactly to this 2-tap box filter
   on S, upsampled 2x with 2x2-constant blocks)

Layout on chip (per image):
  X [128=(32k+g), t(2), ci(4), n(512)] bf16 where n = 2*jj+li2 holds
      img[16g+k+4ci+2li2, 2jj+t]; patch row i = 8g+2ci+li2, channel (k,l),
      l=(t=l%2, dl=l//2): element at [32k+g, t, ci, n+2dl].
  MLP runs with patches as matmul free dim (510 columns = one PSUM bank per
  matmul). Dot-product contractions: per-l products are pre-reduced over l
  (3 bf16 adds on DVE/Pool), then one 128->128 matmul per ci (cw4, ones
  block) contracts k; outputs land at partition 4g+ci = patch-row-pair
  index, so the similarity tile is directly in the fold's layout (no DRAM
  bounce). rsqrt = sqrt activation + reciprocal_approx_fast.
"""

import sys

for _p in ("/opt/trn_rl_repo", "/root/.axon_site/_ro/trn_rl_repo"):
    if _p not in sys.path:
        sys.path.append(_p)

from contextlib import ExitStack

import numpy as np

import concourse.bass as bass
import concourse.tile as tile
from concourse import bacc, mybir

F32 = mybir.dt.float32
BF16 = mybir.dt.bfloat16

IMG = 512
KS = 4
STRIDE = 2
OH = 255  # output patches per dim
NSAMP = 4  # images per core
NCORES = 8

MULT = mybir.AluOpType.mult
ADD = mybir.AluOpType.add
MAX = mybir.AluOpType.max
RELU = mybir.ActivationFunctionType.Relu
SQRT = mybir.ActivationFunctionType.Sqrt
COPY = mybir.ActivationFunctionType.Copy


def build_nc() -> bass.Bass:
    nc = bacc.Bacc()

    img4b = nc.declare_dram_parameter("img4b", [NSAMP, 128, 8 * IMG], BF16, isOutput=False)[:]
    imgsq = nc.declare_dram_parameter("imgsq", [NSAMP, 128, 4 * IMG], BF16, isOutput=False)[:]
    bandw = nc.declare_dram_parameter("bandw", [128, 8, 128], BF16, isOutput=False)[:]
    l1w = nc.declare_dram_parameter("l1w", [128, 4, 128], BF16, isOutput=False)[:]
    l2w = nc.declare_dram_parameter("l2w", [128, 128], BF16, isOutput=False)[:]
    l3w = nc.declare_dram_parameter("l3w", [128, 4, 128], BF16, isOutput=False)[:]
    b3v = nc.declare_dram_parameter("b3v", [128, 4], F32, isOutput=False)[:]
    cw4 = nc.declare_dram_parameter("cw4", [128, 4, 128], BF16, isOutput=False)[:]
    b1v = nc.declare_dram_parameter("b1v", [128, 1], F32, isOutput=False)[:]
    b2v = nc.declare_dram_parameter("b2v", [128, 1], F32, isOutput=False)[:]
    out4 = nc.declare_dram_parameter("out4", [NSAMP, IMG, IMG], F32, isOutput=True)[:]

    with ExitStack() as ctx:
        tc = ctx.enter_context(tile.TileContext(nc))
        consts = ctx.enter_context(tc.tile_pool(name="consts", bufs=1))
        xp = ctx.enter_context(tc.tile_pool(name="xp", bufs=2))
        sqp = ctx.enter_context(tc.tile_pool(name="sqp", bufs=2))
        yvp = ctx.enter_context(tc.tile_pool(name="yvp", bufs=2))
        hp = ctx.enter_context(tc.tile_pool(name="hp", bufs=2))
        prp = ctx.enter_context(tc.tile_pool(name="prp", bufs=2))
        sadd = ctx.enter_context(tc.tile_pool(name="sadd", bufs=1))
        s3p = ctx.enter_context(tc.tile_pool(name="s3p", bufs=2))
        simp = ctx.enter_context(tc.tile_pool(name="simp", bufs=1))
        foldp = ctx.enter_context(tc.tile_pool(name="foldp", bufs=2))
        upp = ctx.enter_context(tc.tile_pool(name="upp", bufs=1))
        psz = ctx.enter_context(tc.tile_pool(name="psz", bufs=1, space="PSUM"))
        psct = ctx.enter_context(tc.tile_pool(name="psct", bufs=1, space="PSUM"))

        # ---- constants ----
        l1w_t = consts.tile([128, 4, 128], BF16)
        nc.sync.dma_start(out=l1w_t, in_=l1w[:, :, :])
        l2w_t = consts.tile([128, 128], BF16)
        nc.sync.dma_start(out=l2w_t, in_=l2w[:, :])
        l3w_t = consts.tile([128, 4, 128], BF16)
        nc.sync.dma_start(out=l3w_t, in_=l3w[:, :, :])
        b3_t = consts.tile([128, 4], F32)
        nc.sync.dma_start(out=b3_t, in_=b3v[:, :])
        cw4_t = consts.tile([128, 4, 128], BF16)
        nc.sync.dma_start(out=cw4_t, in_=cw4[:, :, :])
        bandw_t = consts.tile([128, 8, 128], BF16)
        nc.sync.dma_start(out=bandw_t, in_=bandw[:, :, :])
        b1_t = consts.tile([128, 1], F32)
        nc.sync.dma_start(out=b1_t, in_=b1v[:, :])
        b2_t = consts.tile([128, 1], F32)
        nc.sync.dma_start(out=b2_t, in_=b2v[:, :])
        eps_t = consts.tile([128, 1], F32)
        nc.vector.memset(eps_t, 1e-20)

        def xv(base, l, ci):
            t, dl = l % 2, l // 2
            return base[:, t, ci, 2 * dl : 2 * dl + 510]

        def xva(base, l):
            t, dl = l % 2, l // 2
            return base[:, t, :, 2 * dl : 2 * dl + 510]

        def emit_front(s):
            """DMA + MLP + products + pre-reduce for image s."""
            # ---- input: X in the (t, ci, n) layout; one contiguous DMA ----
            X = xp.tile([128, 2, 4, 512], BF16, tag="x", name=f"X_{s}")
            nc.sync.dma_start(
                out=X, in_=img4b[s].rearrange("p (t c n) -> p t c n", t=2, c=4)
            )

            # |x|^2 path: host-squared natural-layout image; column box-sum on
            # DVE/Pool, then 4-row band-matrix sums on the PE into ctx (tail).
            sqt = sqp.tile([128, 4, 512], BF16, tag="sq", name=f"sqt_{s}")
            nc.sync.dma_start(
                out=sqt, in_=imgsq[s].rearrange("p (t c) -> p t c", t=4)
            )
            sqr = sqt.rearrange("p t (j two) -> p t two j", two=2)
            t1 = sqp.tile([128, 4, 256], BF16, tag="t1", name=f"t1_{s}")
            nc.gpsimd.tensor_tensor(t1, sqr[:, :, 0, :], sqr[:, :, 1, :], ADD)
            bb = sqp.tile([128, 4, 255], BF16, tag="bb", name=f"bb_{s}")
            nc.gpsimd.tensor_tensor(bb, t1[:, :, 0:255], t1[:, :, 1:256], ADD)

            yv = yvp.tile([128, 4, 4, 512], BF16, tag="yv", name=f"yv_{s}")

            # ---- layer 1 (l outer; pair-granularity PSUM tiles, 2 banks each) ----
            z1p = [
                psz.tile([128, 2, 512], F32, tag=f"z{P}", name=f"z1p{P}_{s}")
                for P in range(2)
            ]
            for l in range(4):
                for P in range(2):
                    for ci2 in range(2):
                        nc.tensor.matmul(
                            z1p[P][:, ci2, 0:510], l1w_t[:, l, :],
                            xv(X, l, 2 * P + ci2),
                            start=(l == 0), stop=(l == 3),
                        )
            h1 = []
            for P in range(2):
                h = hp.tile([128, 2, 510], BF16, tag=f"h1{P}", name=f"h1_{P}_{s}")
                nc.scalar.activation(h, z1p[P][:, :, 0:510], RELU, bias=b1_t[:, :])
                h1.append(h)
            # ---- layer 2 ----
            z2p = [
                psz.tile([128, 2, 512], F32, tag=f"z{P}", name=f"z2p{P}_{s}")
                for P in range(2)
            ]
            for P in range(2):
                for ci2 in range(2):
                    nc.tensor.matmul(
                        z2p[P][:, ci2, 0:510], l2w_t[:, :], h1[P][:, ci2, :],
                        start=True, stop=True,
                    )
            h2 = []
            for P in range(2):
                h = hp.tile([128, 2, 510], BF16, tag=f"h2{P}", name=f"h2_{P}_{s}")
                nc.scalar.activation(h, z2p[P][:, :, 0:510], RELU, bias=b2_t[:, :])
                h2.append(h)
            # ---- layer 3: z3 + bias-relu into yv chunks ----
            pr = prp.tile([128, 4, 4, 510], BF16, tag="pr", name=f"pr_{s}")
            for l in range(4):
                for P in range(2):
                    z3 = psz.tile(
                        [128, 2, 512], F32, tag=f"z{P}", name=f"z3_{l}_{P}_{s}"
                    )
                    for ci2 in range(2):
                        nc.tensor.matmul(
                            z3[:, ci2, 0:510], l3w_t[:, l, :], h2[P][:, ci2, :],
                            start=True, stop=True,
                        )
                    nc.scalar.activation(
                        yv[:, 2 * P : 2 * P + 2, l, 0:510], z3[:, :, 0:510],
                        RELU, bias=b3_t[:, l : l + 1],
                    )
                # x*y products for this l over all ci (vector)
                nc.vector.tensor_tensor(pr[:, l], xva(X, l), yv[:, :, l, 0:510], MULT)
            # y^2 in two halves (vector; pool is too slow and contends on SBUF)
            yq = prp.tile([128, 4, 4, 512], BF16, tag="yq", name=f"yq_{s}")
            nc.vector.tensor_tensor(yq[:, 0:2], yv[:, 0:2], yv[:, 0:2], MULT)
            nc.vector.tensor_tensor(yq[:, 2:4], yv[:, 2:4], yv[:, 2:4], MULT)

            # ---- pre-reduce over l (bf16 partial sums, interleaved layout) ----
            s3 = s3p.tile([128, 2, 4, 510], BF16, tag="s3", name=f"s3_{s}")
            with nc.allow_low_precision(reason="bf16 partial dot sums; 2e-2 budget"):
                # dsum (vector)
                da = sadd.tile([128, 4, 510], BF16, tag="da", name=f"da_{s}")
                nc.vector.tensor_tensor(da, pr[:, 0], pr[:, 1], ADD)
                db = sadd.tile([128, 4, 510], BF16, tag="db", name=f"db_{s}")
                nc.vector.tensor_tensor(db, pr[:, 2], pr[:, 3], ADD)
                nc.vector.tensor_tensor(s3[:, 0], da, db, ADD)
                # ysum (pool does the first add)
                ya = sadd.tile([128, 4, 510], BF16, tag="ya", name=f"ya_{s}")
                nc.gpsimd.tensor_tensor(ya, yq[:, :, 0, 0:510], yq[:, :, 1, 0:510], ADD)
                yb = sadd.tile([128, 4, 510], BF16, tag="yb", name=f"yb_{s}")
                nc.vector.tensor_tensor(yb, yq[:, :, 2, 0:510], yq[:, :, 3, 0:510], ADD)
                nc.vector.tensor_tensor(s3[:, 1], ya, yb, ADD)
            return bb, s3

        def emit_tail(s, bb, s3):
            """Contractions + similarity + fold + output for image s."""
            ctd = psct.tile([128, 2, 255], F32, tag="ctd", name=f"ctd_{s}")
            ctx_ = psct.tile([128, 2, 255], F32, tag="ctx", name=f"ctx_{s}")
            cty = psct.tile([128, 2, 255], F32, tag="cty", name=f"cty_{s}")

            # ---- band matmuls: ctx[q',e,:] = sum_k B[4q'+2e+k,:] over 4 tiles ----
            for e in range(2):
                for t in range(4):
                    nc.tensor.matmul(
                        ctx_[:, e, :], bandw_t[:, 2 * t + e, :], bb[:, t, :],
                        start=(t == 0), stop=(t == 3),
                    )

            # ---- contract k: one matmul per (ci, quantity); out partition 4g+ci,
            # moving AP deinterleaves (li2-major cols) so ct/sim/fold layouts match.
            for ci in range(4):
                for q in range(2):
                    nc.tensor.matmul(
                        (ctd, cty)[q],
                        cw4_t[:, ci, :],
                        s3[:, q, ci, :].rearrange("p (j l) -> p l j", l=2),
                        start=(ci == 0), stop=(ci == 3),
                    )

            # ---- cosine similarity (scaled by 1/4 for the fold) ----
            # partition q = 4g+ci = patch-row-pair index (rows 2q+li2): the
            # similarity tile is directly in the fold's row-pair layout.
            ctxs = simp.tile([128, 2, 255], BF16, tag="ctxs")
            nc.scalar.activation(ctxs, ctx_, COPY)
            m_ = simp.tile([128, 2, 255], BF16, tag="m")
            nc.vector.tensor_tensor(m_, ctxs, cty, MULT)
            sq = simp.tile([128, 2, 255], F32, tag="sq")
            nc.scalar.activation(sq, m_, SQRT, bias=eps_t[:, :], scale=16.0)
            r_ = simp.tile([128, 2, 255], F32, tag="r")
            nc.vector.reciprocal_approx_fast(r_, sq)
            simt = foldp.tile([128, 2, 255], BF16, tag="simt")
            nc.vector.tensor_tensor(simt, ctd, r_, MULT)

            # ---- fold (bf16): R[i,v] = S[i,v-1]+S[i,v], with edge doubling ----
            with nc.allow_low_precision(reason="bf16 fold sums; 2e-2 budget"):
                rf = foldp.tile([128, 2, 256], BF16, tag="rf")
                nc.vector.tensor_tensor(
                    rf[:, :, 1:255], simt[:, :, 0:254], simt[:, :, 1:255], ADD
                )
                nc.scalar.activation(rf[:, :, 0:1], simt[:, :, 0:1], COPY, scale=2.0)
                nc.scalar.activation(
                    rf[:, :, 255:256], simt[:, :, 254:255], COPY, scale=2.0
                )
                # S row 255 doesn't exist -> duplicate row 254 so T[255]=2*R[254]
                nc.sync.dma_start(out=rf[127:128, 1, :], in_=rf[127:128, 0, :])
                # partition-shifted copy of odd rows: rfs[q] = R[2q-1] (rfs[0]=R[0])
                rfs = foldp.tile([128, 256], BF16, tag="rfs")
                nc.sync.dma_start(out=rfs[1:128, :], in_=rf[0:127, 1, :])
                nc.sync.dma_start(out=rfs[0:1, :], in_=rf[0:1, 0, :])
                tf = foldp.tile([128, 2, 256], BF16, tag="tf")
                nc.vector.tensor_tensor(tf[:, 1, :], rf[:, 0, :], rf[:, 1, :], ADD)
                nc.vector.tensor_tensor(tf[:, 0, :], rfs, rf[:, 0, :], ADD)

            # ---- upsample: duplicate cols on-chip (2 casts), rows via 2 DMAs ----
            up = upp.tile([128, 2, 512], F32, tag="up")  # (lu, c=2v+cv)
            upr = up.rearrange("p lu (v cv) -> p lu cv v", cv=2)
            nc.gpsimd.tensor_copy(upr[:, :, 0, :], tf[:, :, :])
            nc.vector.tensor_copy(upr[:, :, 1, :], tf[:, :, :])
            for ru in range(2):
                # rows 4q + 2lu + ru <- up[q, lu, :]
                nc.sync.dma_start(
                    out=bass.AP(
                        tensor=out4.tensor,
                        offset=out4.offset + s * IMG * IMG + ru * IMG,
                        ap=[[4 * IMG, 128], [2 * IMG, 2], [1, IMG]],
                    ),
                    in_=up,
                )

        # Software pipeline: each image's contraction/similarity/fold block is
        # emitted after the NEXT image's MLP so the PE never waits on the
        # vector-engine pre-reduce of the image it just fed.
        pending = None
        for s in range(NSAMP):
            front = emit_front(s)
            if pending is not None:
                emit_tail(pending[0], *pending[1])
            pending = (s, front)
        emit_tail(pending[0], *pending[1])

    nc.finalize()
    return nc


def make_weight_inputs(W1, b1, W2, b2, W3, b3):
    """Host-side block-diagonal weight construction (all fp32)."""
    W1 = np.asarray(W1, np.float32)
    W2 = np.asarray(W2, np.float32)
    W3 = np.asarray(W3, np.float32)
    b1 = np.asarray(b1, np.float32)
    b2 = np.asarray(b2, np.float32)
    b3 = np.asarray(b3, np.float32)
    # partition orders: image/z3 rows p = 32k+g ; h1/h2 rows q = 32c+g
    l1w = np.zeros((128, 4, 128), np.float32)
    l2w = np.zeros((128, 128), np.float32)
    l3w = np.zeros((128, 4, 128), np.float32)
    b3v = np.zeros((128, 4), np.float32)
    cw4m = np.zeros((128, 4, 128), np.float32)
    for g in range(32):
        for l in range(4):
            for k in range(4):
                for c in range(4):
                    l1w[32 * k + g, l, 32 * c + g] = W1[4 * k + l, c]
                    l3w[32 * c + g, l, 32 * k + g] = W3[c, 4 * k + l]
                b3v[32 * k + g, l] = b3[4 * k + l]
            for ci in range(4):
                # ct output partition = 4g+ci = patch-row-pair index
                cw4m[32 * l + g, ci, 4 * g + ci] = 1.0
        for c in range(4):
            for c2 in range(4):
                l2w[32 * c + g, 32 * c2 + g] = W2[c, c2]
    b1v = np.repeat(b1, 32).reshape(128, 1).astype(np.float32)
    b2v = np.repeat(b2, 32).reshape(128, 1).astype(np.float32)
    # band weights: bandw[r, 2t+e, m] = 1 iff 4m+2e <= 128t+r <= 4m+2e+3
    bandwm = np.zeros((128, 8, 128), np.float32)
    r = np.arange(128)[:, None]
    m = np.arange(128)[None, :]
    for t in range(4):
        for e in range(2):
            R = 128 * t + r
            bandwm[:, 2 * t + e, :] = (
                (4 * m + 2 * e <= R) & (R <= 4 * m + 2 * e + 3)
            ).astype(np.float32)
    import ml_dtypes

    bf = ml_dtypes.bfloat16
    return {
        "l1w": l1w.astype(bf), "l2w": l2w.astype(bf), "l3w": l3w.astype(bf),
        "b3v": b3v, "cw4": cw4m.astype(bf), "b1v": b1v, "b2v": b2v,
        "bandw": bandwm.astype(bf),
    }


_NC = None


def get_nc():
    global _NC
    if _NC is None:
        _NC = build_nc()
    return _NC


def _bf16():
    import ml_dtypes

    return ml_dtypes.bfloat16


def gather_rows(img_n):
    """(n,512,512) f32 -> (n,128,4096) bf16: X[p,t,ci,jj,li2] = img[16g+k+4ci+2li2, 2jj+t]."""
    n = img_n.shape[0]
    pad = np.zeros((n, IMG + 2, IMG), np.float32)
    pad[:, :IMG, :] = img_n
    p = np.arange(128)
    g, k = p % 32, p // 32
    ci = np.arange(4)
    li2 = np.arange(2)
    t = np.arange(2)
    jj = np.arange(256)
    rows = (
        (16 * g + k)[:, None, None, None, None]
        + 4 * ci[None, None, :, None, None]
        + 2 * li2[None, None, None, None, :]
    )
    cols = (2 * jj[None, None, None, :, None] + t[None, :, None, None, None])
    rows = np.broadcast_to(rows, (128, 2, 4, 256, 2))
    cols = np.broadcast_to(cols, (128, 2, 4, 256, 2))
    out = pad[:, rows, cols]  # (n,128,2,4,256,2)
    return np.ascontiguousarray(out.reshape(n, 128, 8 * IMG)).astype(_bf16())


def gather_sq(img_n):
    """(n,512,512) f32 -> (n,128,2048) bf16: imgsq[p,t,col] = img[128t+p, col]^2."""
    n = img_n.shape[0]
    sq = (img_n.astype(np.float32) ** 2).reshape(n, 4, 128, IMG)
    return np.ascontiguousarray(
        sq.transpose(0, 2, 1, 3).reshape(n, 128, 4 * IMG)
    ).astype(_bf16())


def kernel(img, W1, b1, W2, b2, W3, b3):
    from concourse.bass_utils import run_bass_kernel_spmd

    img = np.asarray(img, np.float32).reshape(32, IMG, IMG)
    wts = make_weight_inputs(W1, b1, W2, b2, W3, b3)
    nc = get_nc()
    core_ids = list(range(NCORES))
    in_maps = []
    for c in range(NCORES):
        m = {
            "img4b": gather_rows(img[c * NSAMP : (c + 1) * NSAMP]),
            "imgsq": gather_sq(img[c * NSAMP : (c + 1) * NSAMP]),
        }
        m.update(wts)
        in_maps.append(m)
    res = run_bass_kernel_spmd(nc, in_maps, core_ids)
    out = np.concatenate([res.results[i]["out4"] for i in range(NCORES)], axis=0)
    return out.astype(np.float32)
